# revision 1
# baseline (speedup 1.0000x reference)
"""Trainium2 Bass kernel for nn_DataAugmentation_46823733461007 (8 NeuronCores).

Reference pipeline per sample: hflip, +0.625*noise, *brightness, perspective
warp (bilinear), rotation warp (bilinear), 190x190 crop, bicubic resize to
224x224.

v2: AB-fold. ap_gather on TRN2 costs ~27ns per index-group regardless of
size, so total index count is the wall. Each 16-partition group (= one
sample) holds TWO row-shifted band copies: partitions 16s+c = rows r0-1..
("A" = y0 taps), partitions 16s+8+c = rows r0.. ("B" = y1 taps). One shared
index slot = y0+1-r0 then serves all four bilinear taps of a pixel in a
single d=2 pair gather (halves gpsimd time vs separate A/B gathers). The
two partition sets are blended with premultiplied fp16 weights and summed
with stride-16 partition-offset DVE adds. Everything else (band DMA, DVE
blends, PE crop+bicubic resize) pipelines under the continuous gather
stream via multi-buffered tile pools.
"""
import sys
sys.path.insert(0, '/opt/trn_rl_repo')
import numpy as np

B, C, H, W = 256, 3, 224, 224
CROP = 190
NCORES = 8
SPC = B // NCORES          # 32 samples per core
ROUNDS = SPC // 8          # 4 rounds x 8 samples
NBAND = 7
RB = H // NBAND            # 32 output rows per band
BR = 108                   # band rows held per variant (107 + 1)
NCH = 4                    # chunks per band
CROWS = RB // NCH          # 8 rows per chunk
NI = CROWS * W             # 1792 indices per chunk per group
NPAIR = W // 2             # 112 pairs per row per parity
NE = 2 * BR * NPAIR        # ap_gather num_elems = 24192


# ------------------------------------------------------------------- host
_XG, _YG = np.meshgrid(np.arange(W, dtype=np.float32) + 0.5,
                       np.arange(H, dtype=np.float32) + 0.5, indexing='xy')


def _persp_coeffs(ep_raw_s):
    offs = np.array([[0., 0.], [195., 0.], [195., 195.], [0., 195.]], np.float32)
    start = np.array([[0., 0.], [223., 0.], [223., 223.], [0., 223.]], np.float32)
    end = ep_raw_s.astype(np.float32) + offs
    ex, ey = end[:, 0], end[:, 1]
    sx, sy = start[:, 0], start[:, 1]
    o = np.ones(4, np.float32); z = np.zeros(4, np.float32)
    r1 = np.stack([ex, ey, o, z, z, z, -sx * ex, -sx * ey], axis=-1)
    r2 = np.stack([z, z, z, ex, ey, o, -sy * ex, -sy * ey], axis=-1)
    A = np.concatenate([r1, r2], axis=0).astype(np.float32)
    b = np.concatenate([sx, sy], axis=0).astype(np.float32)
    return np.linalg.solve(A, b).astype(np.float32)


def persp_grid(ep_raw_s):
    c = _persp_coeffs(ep_raw_s)
    a, b, cc, d, e, f, g, h = [np.float32(c[i]) for i in range(8)]
    den = g * _XG + h * _YG + np.float32(1.0)
    sx = (a * _XG + b * _YG + cc) / den - np.float32(0.5)
    sy = (d * _XG + e * _YG + f) / den - np.float32(0.5)
    return sx.astype(np.float32), sy.astype(np.float32)


def rot_grid(angle):
    th = (np.float32(angle) - np.float32(16.0)) * np.float32(np.pi / 180.0)
    cos = np.float32(np.cos(th)); sin = np.float32(np.sin(th))
    cx = np.float32((W - 1) / 2.0); cy = np.float32((H - 1) / 2.0)
    dx = (_XG - np.float32(0.5)) - cx
    dy = (_YG - np.float32(0.5)) - cy
    rx = (cos * dx + sin * dy + cx).astype(np.float32)
    ry = (-sin * dx + cos * dy + cy).astype(np.float32)
    return rx, ry


def warp_fields(sx, sy):
    """Exact pair-gather decomposition of the reference `_bilinear`."""
    x0 = np.floor(sx); y0 = np.floor(sy)
    wx = (sx - x0).astype(np.float32); wy = (sy - y0).astype(np.float32)
    x0i = x0.astype(np.int64); y0i = y0.astype(np.int64)

    vx0 = ((x0i >= 0) & (x0i < W)).astype(np.float32)
    vx1 = (x0i + 1 < W).astype(np.float32) * (x0i + 1 >= 0)
    vy0 = ((y0i >= 0) & (y0i < H)).astype(np.float32)
    vy1 = (y0i + 1 < H).astype(np.float32) * (y0i + 1 >= 0)

    x0c = np.clip(x0i, 0, W - 1)
    x1c = np.clip(x0i + 1, 0, W - 1)
    px = np.clip(x0i, 0, W - 2)

    w_e0 = np.where(x0c == px, (1 - wx) * vx0, 0.0).astype(np.float32) \
         + np.where(x1c == px, wx * vx1, 0.0).astype(np.float32)
    w_e1 = np.where(x0c == px + 1, (1 - wx) * vx0, 0.0).astype(np.float32) \
         + np.where(x1c == px + 1, wx * vx1, 0.0).astype(np.float32)

    wy0 = ((1 - wy) * vy0).astype(np.float32)
    wy1 = (wy * vy1).astype(np.float32)
    return y0i, px, w_e0, w_e1, wy0, wy1


def band_r0(b):
    return min(max(RB * b - 37, 0), H - 107)


def pack_warp(sx, sy):
    """-> idx: [NBAND*NCH, NI] int16 (shared AB slot index);
       wtA, wtB: [NBAND*NCH, NI, 2] fp16 premultiplied blend weights."""
    y0i, px, w_e0, w_e1, wy0, wy1 = warp_fields(sx, sy)
    eo = (px & 1).astype(np.int64)
    pr = (px >> 1).astype(np.int64)
    ii = np.empty((H, W), np.int64)
    for b in range(NBAND):
        r0 = band_r0(b)
        rs = slice(RB * b, RB * (b + 1))
        slot = np.clip(y0i[rs] + 1 - r0, 0, BR - 1)
        ii[rs] = eo[rs] * (BR * NPAIR) + slot * NPAIR + pr[rs]
    idx = ii.reshape(NBAND * NCH, NI).astype(np.int16)
    wtA = np.stack([wy0 * w_e0, wy0 * w_e1], axis=-1).reshape(NBAND * NCH, NI, 2).astype(np.float16)
    wtB = np.stack([wy1 * w_e0, wy1 * w_e1], axis=-1).reshape(NBAND * NCH, NI, 2).astype(np.float16)
    return idx, wtA, wtB


def wrap16(u):
    """[NI] -> [16, NI//16] wrapped layout for one group."""
    return u.reshape(NI // 16, 16).T


def bicubic_weight_mat(n_in, n_out):
    scale = n_out / n_in

    def kern(x):
        x = np.abs(x); a = -0.5
        return np.where(x <= 1, (a + 2) * x**3 - (a + 3) * x**2 + 1,
                        np.where(x < 2, a * x**3 - 5 * a * x**2 + 8 * a * x - 4 * a, 0.0))

    sample_f = (np.arange(n_out, dtype=np.float64) + 0.5) / scale - 0.5
    x = np.abs(sample_f[None, :] - np.arange(n_in, dtype=np.float64)[:, None])
    wts = kern(x)
    tot = wts.sum(axis=0, keepdims=True)
    wts = np.where(np.abs(tot) > 1000 * np.finfo(np.float32).eps, wts / tot, 0)
    wts = np.where(((sample_f >= -0.5) & (sample_f <= n_in - 0.5))[None, :], wts, 0)
    return wts.astype(np.float32)  # [n_in, n_out]


# ------------------------------------------------------------------ device
_NC_CACHE = [None]


def build_nc():
    import concourse.bacc as bacc
    import concourse.mybir as mybir
    from concourse.tile import TileContext
    fp32, fp16, i16 = mybir.dt.float32, mybir.dt.float16, mybir.dt.int16
    AL = mybir.AluOpType

    nc = bacc.Bacc("TRN2", target_bir_lowering=False, debug=False)

    y16_d = nc.dram_tensor("y16", [ROUNDS, 8, C, 2, H, W], fp16, kind="ExternalInput")
    idx_d = nc.dram_tensor("idxs", [2, ROUNDS, NBAND * NCH, 128, NI // 16], i16, kind="ExternalInput")
    wt_d = nc.dram_tensor("wts", [2, ROUNDS, NBAND * NCH, 2, 8, NI, 2], fp16, kind="ExternalInput")
    selm_d = nc.dram_tensor("selm", [128, 24], fp16, kind="ExternalInput")
    rmov_d = nc.dram_tensor("rmov", [SPC, 112, 2, 2, W], fp16, kind="ExternalInput")
    out_d = nc.dram_tensor("outp", [SPC, C, H, W], fp32, kind="ExternalOutput")
    w1_d = nc.dram_tensor("w1stage", [ROUNDS, 8, C, 2, H, W], fp16)
    w2_d = nc.dram_tensor("w2stage", [ROUNDS, 8, C, H, W], fp16)

    with TileContext(nc) as tc:
        with tc.tile_pool(name="bigp", bufs=1) as bigp, \
             tc.tile_pool(name="smp", bufs=3) as smp, \
             tc.tile_pool(name="rsp", bufs=2) as rsp, \
             tc.tile_pool(name="psp", bufs=2, space="PSUM") as psp:

            bnd = bigp.tile([128, 2, BR, W], fp16, tag="bnd")
            nc.vector.memset(bnd[:], 0.0)
            selm = bigp.tile([128, 24], fp16, tag="selm")
            nc.sync.dma_start(out=selm[:], in_=selm_d[:, :])

            for r in range(ROUNDS):
                for w in range(2):
                    for b in range(NBAND):
                        r0 = band_r0(b)
                        src_d = y16_d if w == 0 else w1_d
                        for v in range(2):
                            lo = r0 - 1 + v
                            s0 = max(lo, 0)
                            s1 = min(lo + BR, H)
                            d0 = s0 - lo
                            nrows = s1 - s0
                            for c in range(C):
                                nc.sync.dma_start(
                                    out=bnd[(8 * v + c)::16, :, d0:d0 + nrows, :],
                                    in_=src_d[r, :, c, :, s0:s1, :])
                        for ch in range(NCH):
                            ci = b * NCH + ch
                            ia = smp.tile([128, NI // 16], i16, tag="ia")
                            nc.sync.dma_start(out=ia[:], in_=idx_d[w, r, ci, :, :])
                            wt = smp.tile([128, NI, 2], fp16, tag="wt")
                            nc.vector.memset(wt[:], 0.0)
                            for c in range(C):
                                nc.sync.dma_start(out=wt[c::16, :, :], in_=wt_d[w, r, ci, 0, :, :, :])
                                nc.sync.dma_start(out=wt[(8 + c)::16, :, :], in_=wt_d[w, r, ci, 1, :, :, :])
                            ga = smp.tile([128, NI, 2], fp16, tag="ga")
                            dat = bnd[:].rearrange("p a b c -> p (a b c)").rearrange("p (n d) -> p n d", d=2)
                            nc.gpsimd.ap_gather(ga[:, :, :], dat, ia[:, :],
                                                channels=128, num_elems=NE, d=2, num_idxs=NI)
                            t1 = smp.tile([128, NI, 2], fp16, tag="t1")
                            nc.vector.tensor_tensor(out=t1[:], in0=ga[:], in1=wt[:], op=AL.mult)
                            s1 = smp.tile([128, NI], fp16, tag="s1")
                            nc.vector.tensor_tensor(out=s1[:], in0=t1[:, :, 0], in1=t1[:, :, 1], op=AL.add)
                            # A+B cross-partition fold: PE matmul vs 0/1
                            # selection matrix; out lands transposed as
                            # [(s,c) partitions, col] so staging DMAs merge.
                            stg = smp.tile([24, CROWS, 2, 112], fp16, tag="stg")
                            for t in range(16):
                                vps = psp.tile([24, 112], fp32, tag="vps")
                                nc.tensor.matmul(
                                    vps[:],
                                    selm[:, :],
                                    s1[:, 112 * t:112 * (t + 1)],
                                    start=True, stop=True)
                                nc.scalar.copy(out=stg[:, t // 2, t % 2, :], in_=vps[:])
                            rr = RB * b + CROWS * ch
                            if w == 0:
                                nc.sync.dma_start(
                                    out=w1_d[r, :, :, 0, rr:rr + CROWS, :].rearrange(
                                        "s c a x -> (s c) (a x)"),
                                    in_=stg[:].rearrange("p a u x -> p (a u x)"))
                                nc.sync.dma_start(
                                    out=w1_d[r, :, :, 1, rr:rr + CROWS, 0:W - 1].rearrange(
                                        "s c a x -> (s c) a x"),
                                    in_=stg[:].rearrange("p a u x -> p a (u x)")[:, :, 1:224])
                            else:
                                nc.sync.dma_start(
                                    out=w2_d[r, :, :, rr:rr + CROWS, :].rearrange(
                                        "s c a x -> (s c) (a x)"),
                                    in_=stg[:].rearrange("p a u x -> p (a u x)"))
                    tc.strict_bb_all_engine_barrier()

                # ---- resize per sample (overlaps next round via rsp/psp bufs)
                for s in range(8):
                    sg = r * 8 + s
                    yrs = rsp.tile([112, 2, C, W], fp16, tag="yrs")
                    for c in range(C):
                        nc.sync.dma_start(
                            out=yrs[:, :, c, :],
                            in_=w2_d[r, s, c, :, :].rearrange("(u p) x -> p u x", u=2))
                    rmv = rsp.tile([112, 2, 2, W], fp16, tag="rmv")
                    nc.sync.dma_start(out=rmv[:], in_=rmov_d[sg, :, :, :, :])
                    for c in range(C):
                        t1t = rsp.tile([112, 2, 224], fp16, tag="t1t")
                        for mh in range(2):
                            acc = psp.tile([112, W], fp32, tag="acc")
                            for kh in range(2):
                                nc.tensor.matmul(
                                    acc[:],
                                    yrs[:, kh, c, mh * 112:(mh + 1) * 112],
                                    rmv[:, 0, kh, :],
                                    start=(kh == 0), stop=(kh == 1))
                            nc.scalar.copy(out=t1t[:, mh, :], in_=acc[:])
                        ost = rsp.tile([112, W], fp32, tag="ost")
                        for mh2 in range(2):
                            acc2 = psp.tile([112, W], fp32, tag="acc2")
                            for kh2 in range(2):
                                nc.tensor.matmul(
                                    acc2[:],
                                    t1t[:, kh2, mh2 * 112:(mh2 + 1) * 112],
                                    rmv[:, 1, kh2, :],
                                    start=(kh2 == 0), stop=(kh2 == 1))
                            nc.scalar.copy(out=ost[:], in_=acc2[:])
                            nc.sync.dma_start(
                                out=out_d[sg, c, mh2 * 112:(mh2 + 1) * 112, :], in_=ost[:])
    nc.compile()
    return nc


# ------------------------------------------------------------------ driver
def _host_pack(inputs):
    x = np.asarray(inputs['x'], np.float32)
    noise = np.asarray(inputs['noise'], np.float32)
    bright = np.asarray(inputs['brightness'], np.float32)
    flip = np.asarray(inputs['flip_mask'], np.int32)
    ep = np.asarray(inputs['ep_raw'], np.int32)
    ang = np.asarray(inputs['angles'], np.int32)
    cij = np.asarray(inputs['crop_ij'], np.int32)

    xf = np.where(flip[:, None, None, None] > 0, x[..., ::-1], x)
    y = (xf + np.float32(0.625) * noise) * (np.float32(0.85) + np.float32(0.30) * bright)[:, None, None, None]
    y16 = y.astype(np.float16)

    R190 = bicubic_weight_mat(CROP, H)     # [190, 224]

    per_core = []
    for core in range(NCORES):
        sl = slice(core * SPC, (core + 1) * SPC)
        ys = y16[sl]                       # [32, 3, 224, 224]
        yy = np.zeros((ROUNDS, 8, C, 2, H, W), np.float16)
        yy[:, :, :, 0] = ys.reshape(ROUNDS, 8, C, H, W)
        yy[:, :, :, 1, :, :W - 1] = ys.reshape(ROUNDS, 8, C, H, W)[..., 1:]

        idx = np.zeros((2, ROUNDS, NBAND * NCH, 128, NI // 16), np.int16)
        wts = np.zeros((2, ROUNDS, NBAND * NCH, 2, 8, NI, 2), np.float16)
        for rr in range(ROUNDS):
            for s in range(8):
                sg = core * SPC + rr * 8 + s
                for w in range(2):
                    if w == 0:
                        sx, sy = persp_grid(ep[sg])
                    else:
                        sx, sy = rot_grid(ang[sg])
                    ii, wA, wB = pack_warp(sx, sy)
                    for ci in range(NBAND * NCH):
                        idx[w, rr, ci, 16 * s:16 * s + 16, :] = wrap16(ii[ci])
                        wts[w, rr, ci, 0, s] = wA[ci]
                        wts[w, rr, ci, 1, s] = wB[ci]

        rmov = np.zeros((SPC, 112, 2, 2, W), np.float16)
        for si in range(SPC):
            sg = core * SPC + si
            i0, j0 = int(cij[sg, 0]), int(cij[sg, 1])
            Rh = np.zeros((H, H), np.float32)
            Rw = np.zeros((H, H), np.float32)
            Rh[i0:i0 + CROP, :] = R190
            Rw[j0:j0 + CROP, :] = R190
            rmov[si, :, 0, :, :] = Rh.reshape(2, 112, W).transpose(1, 0, 2).astype(np.float16)
            rmov[si, :, 1, :, :] = Rw.reshape(2, 112, W).transpose(1, 0, 2).astype(np.float16)

        selm = np.zeros((128, 24), np.float16)
        for s in range(8):
            for c in range(C):
                selm[16 * s + c, 3 * s + c] = 1.0
                selm[16 * s + 8 + c, 3 * s + c] = 1.0

        per_core.append({
            "y16": yy, "idxs": idx, "wts": wts, "rmov": rmov, "selm": selm,
        })
    return per_core


def _axon_shim():
    """Make trace=True work under axon (missing antenv.axon_hooks in image)
    and stub the artifact upload (zero-egress container)."""
    import types
    try:
        import antenv.axon_hooks  # noqa
    except ImportError:
        mod = types.ModuleType('antenv.axon_hooks')
        mod._hook = None
        mod.set_axon_ntff_profile_hook = lambda h: setattr(mod, '_hook', h)
        mod.get_axon_ntff_profile_hook = lambda: mod._hook
        sys.modules['antenv.axon_hooks'] = mod
        import antenv
        antenv.axon_hooks = mod
    from antenv.axon_hooks import get_axon_ntff_profile_hook, set_axon_ntff_profile_hook
    if get_axon_ntff_profile_hook() is None:
        try:
            from trn_agent_boot.trn_boot import _ntff_profile_via_ctypes
            set_axon_ntff_profile_hook(_ntff_profile_via_ctypes('/opt/axon/libaxon_pjrt.so'))
        except Exception:
            pass
    from concourse import bass_utils
    bass_utils.upload_artifacts = lambda tmpdir: f"local://{tmpdir}"


def kernel(**inputs):
    _axon_shim()
    from concourse import bass_utils

    per_core = _host_pack(inputs)
    if _NC_CACHE[0] is None:
        _NC_CACHE[0] = build_nc()
    nc = _NC_CACHE[0]

    import os
    trace = bool(int(os.environ.get("KERNEL_TRACE", "0")))
    res = bass_utils.run_bass_kernel_spmd(
        nc, per_core, list(range(NCORES)), trace=trace)
    if trace and res.exec_time_ns is not None:
        print(f"HW exec time: {res.exec_time_ns} ns")
        kernel.last_exec_ns = res.exec_time_ns
    out = np.concatenate([res.results[i]["outp"] for i in range(NCORES)], axis=0)
    return out.astype(np.float32)



# revision 3
# speedup vs baseline: 1.0043x; 1.0043x over previous
"""Trainium2 Bass kernel for nn_DataAugmentation_46823733461007 (8 NeuronCores).

Reference pipeline per sample: hflip, +0.625*noise, *brightness, perspective
warp (bilinear), rotation warp (bilinear), 190x190 crop, bicubic resize to
224x224.

v2: AB-fold. ap_gather on TRN2 costs ~27ns per index-group regardless of
size, so total index count is the wall. Each 16-partition group (= one
sample) holds TWO row-shifted band copies: partitions 16s+c = rows r0-1..
("A" = y0 taps), partitions 16s+8+c = rows r0.. ("B" = y1 taps). One shared
index slot = y0+1-r0 then serves all four bilinear taps of a pixel in a
single d=2 pair gather (halves gpsimd time vs separate A/B gathers). The
two partition sets are blended with premultiplied fp16 weights and summed
with stride-16 partition-offset DVE adds. Everything else (band DMA, DVE
blends, PE crop+bicubic resize) pipelines under the continuous gather
stream via multi-buffered tile pools.
"""
import sys
sys.path.insert(0, '/opt/trn_rl_repo')
import numpy as np

B, C, H, W = 256, 3, 224, 224
CROP = 190
NCORES = 8
SPC = B // NCORES          # 32 samples per core
ROUNDS = SPC // 8          # 4 rounds x 8 samples
NBAND = 7
RB = H // NBAND            # 32 output rows per band
BR = 108                   # band rows held per variant (107 + 1)
NCH = 4                    # chunks per band
CROWS = RB // NCH          # 8 rows per chunk
NI = CROWS * W             # 1792 indices per chunk per group
NPAIR = W // 2             # 112 pairs per row per parity
NE = 2 * BR * NPAIR        # ap_gather num_elems = 24192


# ------------------------------------------------------------------- host
_XG, _YG = np.meshgrid(np.arange(W, dtype=np.float32) + 0.5,
                       np.arange(H, dtype=np.float32) + 0.5, indexing='xy')


def _persp_coeffs(ep_raw_s):
    offs = np.array([[0., 0.], [195., 0.], [195., 195.], [0., 195.]], np.float32)
    start = np.array([[0., 0.], [223., 0.], [223., 223.], [0., 223.]], np.float32)
    end = ep_raw_s.astype(np.float32) + offs
    ex, ey = end[:, 0], end[:, 1]
    sx, sy = start[:, 0], start[:, 1]
    o = np.ones(4, np.float32); z = np.zeros(4, np.float32)
    r1 = np.stack([ex, ey, o, z, z, z, -sx * ex, -sx * ey], axis=-1)
    r2 = np.stack([z, z, z, ex, ey, o, -sy * ex, -sy * ey], axis=-1)
    A = np.concatenate([r1, r2], axis=0).astype(np.float32)
    b = np.concatenate([sx, sy], axis=0).astype(np.float32)
    return np.linalg.solve(A, b).astype(np.float32)


def persp_grid(ep_raw_s):
    c = _persp_coeffs(ep_raw_s)
    a, b, cc, d, e, f, g, h = [np.float32(c[i]) for i in range(8)]
    den = g * _XG + h * _YG + np.float32(1.0)
    sx = (a * _XG + b * _YG + cc) / den - np.float32(0.5)
    sy = (d * _XG + e * _YG + f) / den - np.float32(0.5)
    return sx.astype(np.float32), sy.astype(np.float32)


def rot_grid(angle):
    th = (np.float32(angle) - np.float32(16.0)) * np.float32(np.pi / 180.0)
    cos = np.float32(np.cos(th)); sin = np.float32(np.sin(th))
    cx = np.float32((W - 1) / 2.0); cy = np.float32((H - 1) / 2.0)
    dx = (_XG - np.float32(0.5)) - cx
    dy = (_YG - np.float32(0.5)) - cy
    rx = (cos * dx + sin * dy + cx).astype(np.float32)
    ry = (-sin * dx + cos * dy + cy).astype(np.float32)
    return rx, ry


def warp_fields(sx, sy):
    """Exact pair-gather decomposition of the reference `_bilinear`."""
    x0 = np.floor(sx); y0 = np.floor(sy)
    wx = (sx - x0).astype(np.float32); wy = (sy - y0).astype(np.float32)
    x0i = x0.astype(np.int64); y0i = y0.astype(np.int64)

    vx0 = ((x0i >= 0) & (x0i < W)).astype(np.float32)
    vx1 = (x0i + 1 < W).astype(np.float32) * (x0i + 1 >= 0)
    vy0 = ((y0i >= 0) & (y0i < H)).astype(np.float32)
    vy1 = (y0i + 1 < H).astype(np.float32) * (y0i + 1 >= 0)

    x0c = np.clip(x0i, 0, W - 1)
    x1c = np.clip(x0i + 1, 0, W - 1)
    px = np.clip(x0i, 0, W - 2)

    w_e0 = np.where(x0c == px, (1 - wx) * vx0, 0.0).astype(np.float32) \
         + np.where(x1c == px, wx * vx1, 0.0).astype(np.float32)
    w_e1 = np.where(x0c == px + 1, (1 - wx) * vx0, 0.0).astype(np.float32) \
         + np.where(x1c == px + 1, wx * vx1, 0.0).astype(np.float32)

    wy0 = ((1 - wy) * vy0).astype(np.float32)
    wy1 = (wy * vy1).astype(np.float32)
    return y0i, px, w_e0, w_e1, wy0, wy1


def band_r0(b):
    return min(max(RB * b - 37, 0), H - 107)


def pack_warp(sx, sy):
    """-> idx: [NBAND*NCH, NI] int16 (shared AB slot index);
       wtA, wtB: [NBAND*NCH, NI, 2] fp16 premultiplied blend weights."""
    y0i, px, w_e0, w_e1, wy0, wy1 = warp_fields(sx, sy)
    eo = (px & 1).astype(np.int64)
    pr = (px >> 1).astype(np.int64)
    ii = np.empty((H, W), np.int64)
    for b in range(NBAND):
        r0 = band_r0(b)
        rs = slice(RB * b, RB * (b + 1))
        slot = np.clip(y0i[rs] + 1 - r0, 0, BR - 1)
        ii[rs] = eo[rs] * (BR * NPAIR) + slot * NPAIR + pr[rs]
    idx = ii.reshape(NBAND * NCH, NI).astype(np.int16)
    wtA = np.stack([wy0 * w_e0, wy0 * w_e1], axis=-1).reshape(NBAND * NCH, NI, 2).astype(np.float16)
    wtB = np.stack([wy1 * w_e0, wy1 * w_e1], axis=-1).reshape(NBAND * NCH, NI, 2).astype(np.float16)
    return idx, wtA, wtB


def wrap16(u):
    """[NI] -> [16, NI//16] wrapped layout for one group."""
    return u.reshape(NI // 16, 16).T


def bicubic_weight_mat(n_in, n_out):
    scale = n_out / n_in

    def kern(x):
        x = np.abs(x); a = -0.5
        return np.where(x <= 1, (a + 2) * x**3 - (a + 3) * x**2 + 1,
                        np.where(x < 2, a * x**3 - 5 * a * x**2 + 8 * a * x - 4 * a, 0.0))

    sample_f = (np.arange(n_out, dtype=np.float64) + 0.5) / scale - 0.5
    x = np.abs(sample_f[None, :] - np.arange(n_in, dtype=np.float64)[:, None])
    wts = kern(x)
    tot = wts.sum(axis=0, keepdims=True)
    wts = np.where(np.abs(tot) > 1000 * np.finfo(np.float32).eps, wts / tot, 0)
    wts = np.where(((sample_f >= -0.5) & (sample_f <= n_in - 0.5))[None, :], wts, 0)
    return wts.astype(np.float32)  # [n_in, n_out]


# ------------------------------------------------------------------ device
_NC_CACHE = [None]


def build_nc():
    import concourse.bacc as bacc
    import concourse.mybir as mybir
    from concourse.tile import TileContext
    fp32, fp16, i16 = mybir.dt.float32, mybir.dt.float16, mybir.dt.int16
    AL = mybir.AluOpType

    nc = bacc.Bacc("TRN2", target_bir_lowering=False, debug=False)

    y16_d = nc.dram_tensor("y16", [ROUNDS, 8, C, 2, H, W], fp16, kind="ExternalInput")
    idx_d = nc.dram_tensor("idxs", [2, ROUNDS, NBAND * NCH, 128, NI // 16], i16, kind="ExternalInput")
    wt_d = nc.dram_tensor("wts", [2, ROUNDS, NBAND * NCH, 2, 8, NI, 2], fp16, kind="ExternalInput")
    selm_d = nc.dram_tensor("selm", [128, 24], fp16, kind="ExternalInput")
    rmov_d = nc.dram_tensor("rmov", [SPC, 112, 2, 2, W], fp16, kind="ExternalInput")
    out_d = nc.dram_tensor("outp", [SPC, C, H, W], fp32, kind="ExternalOutput")
    w1_d = nc.dram_tensor("w1stage", [ROUNDS, 8, C, 2, H, W], fp16)
    w2_d = nc.dram_tensor("w2stage", [ROUNDS, 8, C, H, W], fp16)

    with TileContext(nc) as tc:
        with tc.tile_pool(name="bigp", bufs=1) as bigp, \
             tc.tile_pool(name="smp", bufs=3) as smp, \
             tc.tile_pool(name="rsp", bufs=2) as rsp, \
             tc.tile_pool(name="psp", bufs=2, space="PSUM") as psp:

            bnd = bigp.tile([128, 2, BR, W], fp16, tag="bnd")
            nc.vector.memset(bnd[:], 0.0)
            selm = bigp.tile([128, 24], fp16, tag="selm")
            nc.sync.dma_start(out=selm[:], in_=selm_d[:, :])
            wt_init = 0

            for r in range(ROUNDS):
                for w in range(2):
                    for b in range(NBAND):
                        r0 = band_r0(b)
                        src_d = y16_d if w == 0 else w1_d
                        for v in range(2):
                            lo = r0 - 1 + v
                            s0 = max(lo, 0)
                            s1 = min(lo + BR, H)
                            d0 = s0 - lo
                            nrows = s1 - s0
                            for c in range(C):
                                nc.sync.dma_start(
                                    out=bnd[(8 * v + c)::16, :, d0:d0 + nrows, :],
                                    in_=src_d[r, :, c, :, s0:s1, :])
                        for ch in range(NCH):
                            ci = b * NCH + ch
                            ia = smp.tile([128, NI // 16], i16, tag="ia")
                            nc.sync.dma_start(out=ia[:], in_=idx_d[w, r, ci, :, :])
                            wt = smp.tile([128, NI, 2], fp16, tag="wt")
                            # Unused partitions (3-7, 11-15 of each 16-group)
                            # are never DMA'd; zero each of the 3 pool bufs
                            # once instead of every chunk — removes a DVE
                            # memset from the per-chunk critical chain.
                            if wt_init < 3:
                                nc.vector.memset(wt[:], 0.0)
                                wt_init += 1
                            for c in range(C):
                                nc.sync.dma_start(out=wt[c::16, :, :], in_=wt_d[w, r, ci, 0, :, :, :])
                                nc.sync.dma_start(out=wt[(8 + c)::16, :, :], in_=wt_d[w, r, ci, 1, :, :, :])
                            ga = smp.tile([128, NI, 2], fp16, tag="ga")
                            dat = bnd[:].rearrange("p a b c -> p (a b c)").rearrange("p (n d) -> p n d", d=2)
                            nc.gpsimd.ap_gather(ga[:, :, :], dat, ia[:, :],
                                                channels=128, num_elems=NE, d=2, num_idxs=NI)
                            t1 = smp.tile([128, NI, 2], fp16, tag="t1")
                            nc.vector.tensor_tensor(out=t1[:], in0=ga[:], in1=wt[:], op=AL.mult)
                            s1 = smp.tile([128, NI], fp16, tag="s1")
                            nc.vector.tensor_tensor(out=s1[:], in0=t1[:, :, 0], in1=t1[:, :, 1], op=AL.add)
                            # A+B cross-partition fold: PE matmul vs 0/1
                            # selection matrix; out lands transposed as
                            # [(s,c) partitions, col] so staging DMAs merge.
                            stg = smp.tile([24, CROWS, 2, 112], fp16, tag="stg")
                            for t in range(16):
                                vps = psp.tile([24, 112], fp32, tag="vps")
                                nc.tensor.matmul(
                                    vps[:],
                                    selm[:, :],
                                    s1[:, 112 * t:112 * (t + 1)],
                                    start=True, stop=True)
                                nc.scalar.copy(out=stg[:, t // 2, t % 2, :], in_=vps[:])
                            rr = RB * b + CROWS * ch
                            if w == 0:
                                nc.sync.dma_start(
                                    out=w1_d[r, :, :, 0, rr:rr + CROWS, :].rearrange(
                                        "s c a x -> (s c) (a x)"),
                                    in_=stg[:].rearrange("p a u x -> p (a u x)"))
                                nc.sync.dma_start(
                                    out=w1_d[r, :, :, 1, rr:rr + CROWS, 0:W - 1].rearrange(
                                        "s c a x -> (s c) a x"),
                                    in_=stg[:].rearrange("p a u x -> p a (u x)")[:, :, 1:224])
                            else:
                                nc.sync.dma_start(
                                    out=w2_d[r, :, :, rr:rr + CROWS, :].rearrange(
                                        "s c a x -> (s c) (a x)"),
                                    in_=stg[:].rearrange("p a u x -> p (a u x)"))
                    tc.strict_bb_all_engine_barrier()

                # ---- resize per sample (overlaps next round via rsp/psp bufs)
                for s in range(8):
                    sg = r * 8 + s
                    yrs = rsp.tile([112, 2, C, W], fp16, tag="yrs")
                    for c in range(C):
                        nc.sync.dma_start(
                            out=yrs[:, :, c, :],
                            in_=w2_d[r, s, c, :, :].rearrange("(u p) x -> p u x", u=2))
                    rmv = rsp.tile([112, 2, 2, W], fp16, tag="rmv")
                    nc.sync.dma_start(out=rmv[:], in_=rmov_d[sg, :, :, :, :])
                    for c in range(C):
                        t1t = rsp.tile([112, 2, 224], fp16, tag="t1t")
                        for mh in range(2):
                            acc = psp.tile([112, W], fp32, tag="acc")
                            for kh in range(2):
                                nc.tensor.matmul(
                                    acc[:],
                                    yrs[:, kh, c, mh * 112:(mh + 1) * 112],
                                    rmv[:, 0, kh, :],
                                    start=(kh == 0), stop=(kh == 1))
                            nc.scalar.copy(out=t1t[:, mh, :], in_=acc[:])
                        ost = rsp.tile([112, W], fp32, tag="ost")
                        for mh2 in range(2):
                            acc2 = psp.tile([112, W], fp32, tag="acc2")
                            for kh2 in range(2):
                                nc.tensor.matmul(
                                    acc2[:],
                                    t1t[:, kh2, mh2 * 112:(mh2 + 1) * 112],
                                    rmv[:, 1, kh2, :],
                                    start=(kh2 == 0), stop=(kh2 == 1))
                            nc.scalar.copy(out=ost[:], in_=acc2[:])
                            nc.sync.dma_start(
                                out=out_d[sg, c, mh2 * 112:(mh2 + 1) * 112, :], in_=ost[:])
    nc.compile()
    return nc


# ------------------------------------------------------------------ driver
def _host_pack(inputs):
    x = np.asarray(inputs['x'], np.float32)
    noise = np.asarray(inputs['noise'], np.float32)
    bright = np.asarray(inputs['brightness'], np.float32)
    flip = np.asarray(inputs['flip_mask'], np.int32)
    ep = np.asarray(inputs['ep_raw'], np.int32)
    ang = np.asarray(inputs['angles'], np.int32)
    cij = np.asarray(inputs['crop_ij'], np.int32)

    xf = np.where(flip[:, None, None, None] > 0, x[..., ::-1], x)
    y = (xf + np.float32(0.625) * noise) * (np.float32(0.85) + np.float32(0.30) * bright)[:, None, None, None]
    y16 = y.astype(np.float16)

    R190 = bicubic_weight_mat(CROP, H)     # [190, 224]

    per_core = []
    for core in range(NCORES):
        sl = slice(core * SPC, (core + 1) * SPC)
        ys = y16[sl]                       # [32, 3, 224, 224]
        yy = np.zeros((ROUNDS, 8, C, 2, H, W), np.float16)
        yy[:, :, :, 0] = ys.reshape(ROUNDS, 8, C, H, W)
        yy[:, :, :, 1, :, :W - 1] = ys.reshape(ROUNDS, 8, C, H, W)[..., 1:]

        idx = np.zeros((2, ROUNDS, NBAND * NCH, 128, NI // 16), np.int16)
        wts = np.zeros((2, ROUNDS, NBAND * NCH, 2, 8, NI, 2), np.float16)
        for rr in range(ROUNDS):
            for s in range(8):
                sg = core * SPC + rr * 8 + s
                for w in range(2):
                    if w == 0:
                        sx, sy = persp_grid(ep[sg])
                    else:
                        sx, sy = rot_grid(ang[sg])
                    ii, wA, wB = pack_warp(sx, sy)
                    for ci in range(NBAND * NCH):
                        idx[w, rr, ci, 16 * s:16 * s + 16, :] = wrap16(ii[ci])
                        wts[w, rr, ci, 0, s] = wA[ci]
                        wts[w, rr, ci, 1, s] = wB[ci]

        rmov = np.zeros((SPC, 112, 2, 2, W), np.float16)
        for si in range(SPC):
            sg = core * SPC + si
            i0, j0 = int(cij[sg, 0]), int(cij[sg, 1])
            Rh = np.zeros((H, H), np.float32)
            Rw = np.zeros((H, H), np.float32)
            Rh[i0:i0 + CROP, :] = R190
            Rw[j0:j0 + CROP, :] = R190
            rmov[si, :, 0, :, :] = Rh.reshape(2, 112, W).transpose(1, 0, 2).astype(np.float16)
            rmov[si, :, 1, :, :] = Rw.reshape(2, 112, W).transpose(1, 0, 2).astype(np.float16)

        selm = np.zeros((128, 24), np.float16)
        for s in range(8):
            for c in range(C):
                selm[16 * s + c, 3 * s + c] = 1.0
                selm[16 * s + 8 + c, 3 * s + c] = 1.0

        per_core.append({
            "y16": yy, "idxs": idx, "wts": wts, "rmov": rmov, "selm": selm,
        })
    return per_core


def _axon_shim():
    """Make trace=True work under axon (missing antenv.axon_hooks in image)
    and stub the artifact upload (zero-egress container)."""
    import types
    try:
        import antenv.axon_hooks  # noqa
    except ImportError:
        mod = types.ModuleType('antenv.axon_hooks')
        mod._hook = None
        mod.set_axon_ntff_profile_hook = lambda h: setattr(mod, '_hook', h)
        mod.get_axon_ntff_profile_hook = lambda: mod._hook
        sys.modules['antenv.axon_hooks'] = mod
        import antenv
        antenv.axon_hooks = mod
    from antenv.axon_hooks import get_axon_ntff_profile_hook, set_axon_ntff_profile_hook
    if get_axon_ntff_profile_hook() is None:
        try:
            from trn_agent_boot.trn_boot import _ntff_profile_via_ctypes
            set_axon_ntff_profile_hook(_ntff_profile_via_ctypes('/opt/axon/libaxon_pjrt.so'))
        except Exception:
            pass
    from concourse import bass_utils
    bass_utils.upload_artifacts = lambda tmpdir: f"local://{tmpdir}"


def kernel(**inputs):
    _axon_shim()
    from concourse import bass_utils

    per_core = _host_pack(inputs)
    if _NC_CACHE[0] is None:
        _NC_CACHE[0] = build_nc()
    nc = _NC_CACHE[0]

    import os
    trace = bool(int(os.environ.get("KERNEL_TRACE", "0")))
    res = bass_utils.run_bass_kernel_spmd(
        nc, per_core, list(range(NCORES)), trace=trace)
    if trace and res.exec_time_ns is not None:
        print(f"HW exec time: {res.exec_time_ns} ns")
        kernel.last_exec_ns = res.exec_time_ns
    out = np.concatenate([res.results[i]["outp"] for i in range(NCORES)], axis=0)
    return out.astype(np.float32)

